# revision 1
# baseline (speedup 1.0000x reference)
"""DechirpSTFT Trainium2 kernel (8 NeuronCores).

Math: out[d,b,w,:] = FFT_1024(chirp * resample_d(hann * window(x[b], w)))

Factorization per (d, b):
  - window + hann + linear-interp resample  ->  banded matrix G_d applied by
    TensorE directly to x held in SBUF as [128, 4096] (window = stride-4
    column slice; hop 512 = 4 cols of 128).  G's columns emit y in radix-2
    DIT order: z-tile (n2, t) holds y[2*(128 t + p) + n2].
  - radix-2 DIT FFT (1024 = 512 x 2): stage-1 = two 512-point complex DFT
    matrices M_{n2}[n1,k1] = chirp[2 n1 + n2] * W512^{n1 k1} * W1024^{n2 k1}
    (chirp + twiddle folded in), applied by TensorE (contraction over n1).
  - tail: Y[k1] = V0 + V1, Y[k1+512] = V0 - V1 on VectorE (doubles as the
    PSUM evacuation), written re/im-interleaved for a contiguous output DMA.
All matmuls in float32r (1 cyc/row at N>=256, ~1.5e-4 rel err).
Each core owns 2 of the 16 chirp rates.
"""

import numpy as np

K = 1024
HOP = 512
CHIRP_A = 0.5
NB = 2
NX = 524288
W = (NX - K) // HOP + 1          # 1023
D = 16
NCORES = 8
DLOC = D // NCORES               # 2 chirp rates per core
K1, K2 = 512, 2
WT = 512                          # windows per chunk (matmul moving dim)
NWC = 2                           # ceil(1023/512)
NSLOT = 3                         # interp source tiles per z-tile (x shifted by -64)
XCOLS = 4104                      # 4096 cols + pad so window 1023 reads zeros

_NC_CACHE = {}
_LAST_RESULTS = {}
_REPEAT = 1  # >1: wrap body in a device-side loop (timing experiments only)
_VARIANT = "full"  # timing-only: full | nodma | notail | mmonly


def _host_tables_all(dlnf):
    """(16,) -> lo (D,K) int32, frac (D,K) f32.  Computed with jax on CPU,
    bit-exactly mirroring reference.py's fp32 pipeline (numpy's fp32
    exp/log1p differ from XLA's by enough to shift idx by ~1e-3 samples)."""
    import jax
    import jax.numpy as jnp

    cpu = jax.devices("cpu")[0]
    with jax.default_device(cpu):
        betas = 2.0 * jnp.asarray(np.asarray(dlnf, dtype=np.float32))
        safe = jnp.abs(betas) < 1e-8
        bs = jnp.where(safe, jnp.float32(1e-8), betas)
        tau = jnp.linspace(0.0, 1.0, K, dtype=jnp.float32)
        t_src = 2.0 / bs[:, None] * jnp.log1p(
            tau[None, :] * (jnp.exp(bs)[:, None] - 1.0)) - 1.0
        identity = jnp.linspace(-1.0, 1.0, K, dtype=jnp.float32)
        t_src = jnp.where(safe[:, None], identity[None, :], t_src)
        idx = (t_src + 1.0) * 0.5 * (K - 1)
        lo = jnp.clip(idx.astype(jnp.int32), 0, K - 2)
        frac = idx - lo.astype(idx.dtype)
    return np.asarray(lo), np.asarray(frac).astype(np.float32)


def _jt_slots(t):
    """Source x j-tile slots for z-tile t, on the 64-sample-shifted x grid
    (tile m covers j in [128 m - 64, 128 m + 64)); same for all d."""
    return [2 * t, 2 * t + 1, 2 * t + 2]


def _build_g(lo_pair, frac_pair):
    """Interp stationaries, packed [128, 2*2*4*4*128] fp32.
    Col block ((d2*2+n2)*4+t)*4+s holds G[q, p]: src j=128*jt+q -> n=256t+2p+n2."""
    hann = (0.5 * (1.0 - np.cos(2.0 * np.pi * np.arange(K) / K))).astype(np.float32)
    g = np.zeros((128, DLOC * 2 * 4 * NSLOT * 128), dtype=np.float32)
    nn = np.arange(K)
    n2a, nh = nn & 1, nn >> 1
    ta, pa = nh >> 7, nh & 127
    for d2 in range(DLOC):
        lo = lo_pair[d2]
        frac = frac_pair[d2]
        alpha = ((1.0 - frac) * hann[lo]).astype(np.float32)
        beta = (frac * hann[lo + 1]).astype(np.float32)
        for j, val in ((lo, alpha), (lo + 1, beta)):
            m, q = (j + 64) >> 7, (j + 64) & 127
            s = m - 2 * ta
            if not np.all((s >= 0) & (s < NSLOT)):
                raise ValueError("interp band exceeds the 3 source-tile slots")
            flat = ((d2 * 2 + n2a) * 4 + ta) * NSLOT + s
            np.add.at(g, (q, flat * 128 + pa), val)
    return g


def _build_m1():
    """Stage-1 DFT stationaries [128, 2*2*4*4*128] fp32 (d-independent).
    Col block ((n2*2+pl)*4+kt)*4+mc holds M[q, c]: n1=128kt+q, k1=128mc+c."""
    t_norm = np.linspace(-1.0, 1.0, K).astype(np.float64)
    chirp = np.exp(-1j * CHIRP_A * t_norm ** 2)
    m1 = np.zeros((128, 2 * 2 * 4 * 4 * 128), dtype=np.float32)
    n1g = np.arange(K1)
    k1g = np.arange(K1)
    for n2 in range(2):
        M = (chirp[2 * n1g + n2][:, None]
             * np.exp(-2j * np.pi * np.outer(n1g, k1g) / K1)
             * np.exp(-2j * np.pi * n2 * k1g / K)[None, :])
        for pl in range(2):
            plane = (M.real if pl == 0 else M.imag).astype(np.float32)
            for kt in range(4):
                for mc in range(4):
                    flat = ((mc * 2 + n2) * 2 + pl) * 4 + kt
                    m1[:, flat * 128:(flat + 1) * 128] = \
                        plane[128 * kt:128 * kt + 128, 128 * mc:128 * mc + 128]
    return m1


def _build_program():
    import concourse.bacc as bacc
    import concourse.mybir as mybir
    from concourse.tile import TileContext

    f32 = mybir.dt.float32
    f32r = mybir.dt.float32r

    nc = bacc.Bacc("TRN2", target_bir_lowering=False, debug=False,
                   num_devices=NCORES)
    xT = nc.dram_tensor("xT", [NB, 128, XCOLS], f32r, kind="ExternalInput")
    g = nc.dram_tensor("g", [128, DLOC * 2 * 4 * NSLOT * 128], f32r,
                       kind="ExternalInput")
    m1 = nc.dram_tensor("m1", [128, 2 * 2 * 4 * 4 * 128], f32r,
                        kind="ExternalInput")
    out_t = nc.dram_tensor("out", [DLOC, NB, K, W, 2], f32,
                           kind="ExternalOutput")

    def gcol(d2, n2, t, s):
        flat = ((d2 * 2 + n2) * 4 + t) * NSLOT + s
        return slice(flat * 128, (flat + 1) * 128)

    def m1col(n2, pl, kt, mc):
        flat = ((mc * 2 + n2) * 2 + pl) * 4 + kt
        return slice(flat * 128, (flat + 1) * 128)

    with TileContext(nc) as tc:
        with (
            tc.tile_pool(name="resident", bufs=1) as rp,
            tc.tile_pool(name="ysb", bufs=8) as yp,
            tc.tile_pool(name="osb", bufs=4) as op,
            tc.tile_pool(name="py", bufs=2, space="PSUM") as pyp,
            tc.tile_pool(name="pv", bufs=2, space="PSUM") as pvp,
        ):
            # split resident loads so early compute overlaps later slices
            xt_sb = []
            for b in range(NB):
                xb = rp.tile([128, XCOLS], f32r, tag=f"x{b}")
                cut = 4 * (WT - 1) + 9       # cols needed by chunk wc=0
                nc.sync.dma_start(out=xb[:, 0:cut], in_=xT[b, :, 0:cut])
                nc.sync.dma_start(out=xb[:, cut:], in_=xT[b, :, cut:])
                xt_sb.append(xb)
            g_sb = rp.tile([128, DLOC * 2 * 4 * NSLOT * 128], f32r, tag="g")
            gh = DLOC * 2 * 4 * NSLOT * 128 // 2
            nc.sync.dma_start(out=g_sb[:, 0:gh], in_=g[:, 0:gh])
            nc.sync.dma_start(out=g_sb[:, gh:], in_=g[:, gh:])
            m1_sb = rp.tile([128, 2 * 2 * 4 * 4 * 128], f32r, tag="m1")
            m1q = 2 * 2 * 4 * 4 * 128 // 4
            for qq in range(4):
                nc.sync.dma_start(out=m1_sb[:, qq * m1q:(qq + 1) * m1q],
                                  in_=m1[:, qq * m1q:(qq + 1) * m1q])

            def emit_interp(d2, b, wc):
                """interp/gather matmuls + ACT evac; returns the 4 y tiles."""
                w0 = WT * wc
                ytiles = []
                for t in range(4):
                    py = pyp.tile([128, 2 * WT], f32, tag="py")
                    for n2 in range(2):
                        dst = py[:, n2 * WT:(n2 + 1) * WT]
                        for s, m in enumerate(_jt_slots(t)):
                            base = 4 * w0 + m
                            rhs = xt_sb[b][:, base:base + 4 * WT:4]
                            nc.tensor.matmul(
                                dst, g_sb[:, gcol(d2, n2, t, s)], rhs,
                                start=(s == 0), stop=(s == NSLOT - 1))
                    ysb = yp.tile([128, 2 * WT], f32r, tag="y")
                    if _VARIANT != "mmonly":
                        nc.scalar.copy(ysb[:, :], py[:, :])
                    ytiles.append(ysb)
                return ytiles

            def emit_stage1(d2, b, wc, ytiles):
                w0 = WT * wc
                wn = min(WT, W - w0)
                for mc in range(4):
                    pv = []
                    v0s = []
                    for pl in range(2):
                        pvt = pvp.tile([128, 2 * WT], f32, tag="pv")
                        for n2 in range(2):
                            dst = pvt[:, n2 * WT:(n2 + 1) * WT]
                            for kt in range(4):
                                nc.tensor.matmul(
                                    dst,
                                    m1_sb[:, m1col(n2, pl, kt, mc)],
                                    ytiles[kt][:, n2 * WT:(n2 + 1) * WT],
                                    start=(kt == 0), stop=(kt == 3))
                        pv.append(pvt)
                        # DVE ops may read only one PSUM operand:
                        # stage V0 (the n2=0 half) through SBUF.
                        if _VARIANT in ("full", "nodma"):
                            v0t = yp.tile([128, WT], f32, tag="v0")
                            nc.scalar.copy(v0t[:, :], pvt[:, 0:WT])
                            v0s.append(v0t)
                    if _VARIANT in ("full", "nodma"):
                        for k2 in range(2):
                            ot = op.tile([128, 2 * WT], f32, tag="o")
                            for pl in range(2):
                                dst = ot[:, pl:2 * WT:2]
                                v0 = v0s[pl][:, :]
                                v1 = pv[pl][:, WT:2 * WT]
                                if k2 == 0:
                                    nc.vector.tensor_add(dst, v0, v1)
                                else:
                                    nc.vector.tensor_sub(dst, v0, v1)
                            if _VARIANT == "full":
                                kb = 128 * mc + 512 * k2
                                nc.sync.dma_start(
                                    out=out_t[d2, b, kb:kb + 128, w0:w0 + wn, :],
                                    in_=ot[:, 0:2 * wn].rearrange(
                                        "p (w r) -> p w r", r=2))
                    elif _VARIANT == "notail":
                        ot = op.tile([128, 2 * WT], f32, tag="o")
                        nc.vector.tensor_copy(ot[:, 0:WT], pv[0][:, 0:WT])

            import contextlib
            import os as _os
            _hints = ()
            if _os.environ.get("LOOP_HINTS"):
                _hints = (mybir.EngineType.PE, mybir.EngineType.Activation,
                          mybir.EngineType.DVE, mybir.EngineType.SP)
            rep_ctx = (tc.For_i(0, _REPEAT, 1, hint_engines=_hints)
                       if _REPEAT > 1 else contextlib.nullcontext())
            with rep_ctx:
                # software pipeline: interp(i+1) issues before stage1(i) so the
                # PE never waits on the interp->ACT-evac->stage1 chain
                iters = [(d2, b, wc) for d2 in range(DLOC)
                         for b in range(NB) for wc in range(NWC)]
                pending = emit_interp(*iters[0])
                for i, it in enumerate(iters):
                    nxt = (emit_interp(*iters[i + 1])
                           if i + 1 < len(iters) else None)
                    emit_stage1(*it, pending)
                    pending = nxt
    nc.compile()
    return nc


def _host_prep(x, dlnf):
    x = np.ascontiguousarray(np.asarray(x, dtype=np.float32))
    dlnf = np.asarray(dlnf, dtype=np.float32)
    # x shifted by -64 into partition-interleaved layout:
    # xT[b, q, c] = x[b, 128 c + q - 64]  (zeros outside [0, NX))
    xT = np.zeros((NB, 128, XCOLS), dtype=np.float32)
    xs = np.zeros((NB, XCOLS * 128), dtype=np.float32)
    xs[:, 64:64 + NX] = x
    xT[:, :, :] = np.transpose(xs.reshape(NB, XCOLS, 128), (0, 2, 1))
    m1 = _build_m1()
    lo_all, frac_all = _host_tables_all(dlnf)
    in_maps = []
    for c in range(NCORES):
        gc_ = _build_g(lo_all[DLOC * c: DLOC * (c + 1)],
                       frac_all[DLOC * c: DLOC * (c + 1)])
        in_maps.append({"xT": xT, "g": gc_, "m1": m1})
    return in_maps


def kernel(x, dlnf):
    from concourse.bass_utils import run_bass_kernel_spmd

    in_maps = _host_prep(x, dlnf)
    if "nc" not in _NC_CACHE:
        _NC_CACHE["nc"] = _build_program()
    nc = _NC_CACHE["nc"]
    res = run_bass_kernel_spmd(nc, in_maps, core_ids=list(range(NCORES)))
    _LAST_RESULTS["res"] = res
    outs = []
    for c in range(NCORES):
        o = res.results[c]["out"]                      # [DLOC, NB, K, W, 2] f32
        cplx = (o[..., 0] + 1j * o[..., 1]).astype(np.complex64)
        outs.append(np.transpose(cplx, (0, 1, 3, 2)))  # -> [DLOC, NB, W, K]
    return np.concatenate(outs, axis=0)



# revision 23
# speedup vs baseline: 1.2280x; 1.2280x over previous
"""DechirpSTFT Trainium2 kernel (8 NeuronCores).

Math: out[d,b,w,:] = FFT_1024(chirp * resample_d(hann * window(x[b], w)))

Per (d, b), radix-4 DIT with every twiddle folded into a stationary:
  - window + hann + linear-interp resample -> banded matrix G_d applied by
    TensorE to x held in SBUF as [128, 4104] bf16 (window = stride-4 column
    slice; hop 512 = 4 cols of 128).  G emits z-tiles (r, mt) holding
    y[4*(128*mt + p) + r]; each needs 5 source-tile slots (n-span 512).
  - stage: T_r[u] = sum_m chirp[4m+r] W1024^{r u} W256^{m u} y[4m+r],
    u in [0,256) -- four 256-point DFTs (chirp + r-twiddle folded into the
    stationaries), TensorE, 2 real planes, contraction 256 = 2 k-tiles.
  - combine (multiply-free, DVE bf16): A=T0+T2, B=T0-T2, C=T1+T3, U=T1-T3;
    out[q=0]=A+C, out[q=2]=A-C, out[q=1]=(Bre+Uim, Bim-Ure),
    out[q=3]=(Bre-Uim, Bim+Ure), written as packed fp16 re/im planes
    (host interleaves to complex64).  72 matmuls/iter vs 88 for the
    radix-2 dense-512 scheme.
Each core owns 2 of the 16 chirp rates.
"""

import os as _os

import ml_dtypes
import numpy as np

K = 1024
HOP = 512
CHIRP_A = 0.5
NB = 2
NX = 524288
W = (NX - K) // HOP + 1          # 1023
D = 16
NCORES = 8
DLOC = D // NCORES               # 2 chirp rates per core
WT = 512                          # windows per chunk (matmul moving dim)
NWC = 2                           # ceil(1023/512)
NSLOT = 5                         # interp source tiles per z-tile
XCOLS = 4104                      # 4096 cols + pad so window 1023 reads zeros
GCOLS = DLOC * 4 * 2 * NSLOT * 128
MCOLS = 4 * 2 * 2 * 2 * 128

_NC_CACHE = {}
_LAST_RESULTS = {}
_REPEAT = 1  # >1: wrap body in a device-side loop (timing experiments only)
_VARIANT = "full"  # timing-only: full | nodma | notail | mmonly
_TIMING = False  # timing-only: out stays in DRAM (Internal); tiny tick output


def _host_tables_all(dlnf):
    """(16,) -> lo (D,K) int32, frac (D,K) f32.  Computed with jax on CPU,
    bit-exactly mirroring reference.py's fp32 pipeline (numpy's fp32
    exp/log1p differ from XLA's by enough to shift idx by ~1e-3 samples)."""
    import jax
    import jax.numpy as jnp

    cpu = jax.devices("cpu")[0]
    with jax.default_device(cpu):
        betas = 2.0 * jnp.asarray(np.asarray(dlnf, dtype=np.float32))
        safe = jnp.abs(betas) < 1e-8
        bs = jnp.where(safe, jnp.float32(1e-8), betas)
        tau = jnp.linspace(0.0, 1.0, K, dtype=jnp.float32)
        t_src = 2.0 / bs[:, None] * jnp.log1p(
            tau[None, :] * (jnp.exp(bs)[:, None] - 1.0)) - 1.0
        identity = jnp.linspace(-1.0, 1.0, K, dtype=jnp.float32)
        t_src = jnp.where(safe[:, None], identity[None, :], t_src)
        idx = (t_src + 1.0) * 0.5 * (K - 1)
        lo = jnp.clip(idx.astype(jnp.int32), 0, K - 2)
        frac = idx - lo.astype(idx.dtype)
    return np.asarray(lo), np.asarray(frac).astype(np.float32)


def _build_g(lo_pair, frac_pair):
    """Interp stationaries, packed [128, DLOC*4*2*5*128] fp32.
    Col block ((d2*4+r)*2+mt)*5+s holds G[q, p]:
    src x[128*(4mt+s) + q - 64]  ->  y[4*(128*mt + p) + r]."""
    hann = (0.5 * (1.0 - np.cos(2.0 * np.pi * np.arange(K) / K))).astype(
        np.float32)
    g = np.zeros((128, GCOLS), dtype=np.float32)
    nn = np.arange(K)
    r_, mh = nn & 3, nn >> 2
    mt_, p_ = mh >> 7, mh & 127
    for d2 in range(DLOC):
        lo = lo_pair[d2]
        frac = frac_pair[d2]
        alpha = ((1.0 - frac) * hann[lo]).astype(np.float32)
        beta = (frac * hann[lo + 1]).astype(np.float32)
        for j, val in ((lo, alpha), (lo + 1, beta)):
            m, q = (j + 64) >> 7, (j + 64) & 127
            s = m - 4 * mt_
            if not np.all((s >= 0) & (s < NSLOT)):
                raise ValueError("interp band exceeds the source-tile slots")
            flat = ((d2 * 4 + r_) * 2 + mt_) * NSLOT + s
            np.add.at(g, (q, flat * 128 + p_), val)
    return g


def _build_m2():
    """Stage stationaries [128, 4*2*2*2*128] fp32 (d-independent).
    Col block ((r*2+pl)*2+kt)*2+j holds plane pl of
    M_r[m, u] = chirp[4m+r] * W256^{m u} * W1024^{r u},
    rows m = 128*kt + row, cols u = 128*j + col."""
    t_norm = np.linspace(-1.0, 1.0, K).astype(np.float64)
    chirp = np.exp(-1j * CHIRP_A * t_norm ** 2)
    m2 = np.zeros((128, MCOLS), dtype=np.float32)
    mg = np.arange(256)
    ug = np.arange(256)
    for r in range(4):
        M = (chirp[4 * mg + r][:, None]
             * np.exp(-2j * np.pi * np.outer(mg, ug) / 256.0)
             * np.exp(-2j * np.pi * r * ug / 1024.0)[None, :])
        for pl in range(2):
            plane = (M.real if pl == 0 else M.imag).astype(np.float32)
            for kt in range(2):
                for j in range(2):
                    flat = ((r * 2 + pl) * 2 + kt) * 2 + j
                    m2[:, flat * 128:(flat + 1) * 128] = \
                        plane[128 * kt:128 * kt + 128, 128 * j:128 * j + 128]
    return m2


def _build_program():
    import concourse.bacc as bacc
    import concourse.mybir as mybir
    from concourse.tile import TileContext

    f32 = mybir.dt.float32
    bf16 = mybir.dt.bfloat16
    fp16 = mybir.dt.float16

    nc = bacc.Bacc("TRN2", target_bir_lowering=False, debug=False,
                   num_devices=NCORES)
    xT = nc.dram_tensor("xT", [NB, 128, XCOLS], bf16, kind="ExternalInput")
    g = nc.dram_tensor("g", [128, GCOLS], bf16, kind="ExternalInput")
    m1 = nc.dram_tensor("m1", [128, MCOLS], bf16, kind="ExternalInput")
    # output: separate re/im planes, W padded to 1024 so DMAs are full-tile
    out_t = nc.dram_tensor("out", [DLOC, NB, K, 2, 1024], fp16,
                           kind="Internal" if _TIMING else "ExternalOutput")
    tick = (nc.dram_tensor("tick", [128, 1], bf16, kind="ExternalOutput")
            if _TIMING else None)

    def gcol(d2, r, mt, s):
        flat = ((d2 * 4 + r) * 2 + mt) * NSLOT + s
        return slice(flat * 128, (flat + 1) * 128)

    def m2col(r, pl, kt, j):
        flat = ((r * 2 + pl) * 2 + kt) * 2 + j
        return slice(flat * 128, (flat + 1) * 128)

    with TileContext(nc) as tc:
        with (
            tc.tile_pool(name="resident", bufs=1) as rp,
            tc.tile_pool(name="ysb", bufs=18) as yp,
            tc.tile_pool(name="tsb", bufs=12) as tp,
            tc.tile_pool(name="csb", bufs=10) as cp,
            tc.tile_pool(name="osb", bufs=4) as op,
            tc.tile_pool(name="py", bufs=2, space="PSUM") as pyp,
            tc.tile_pool(name="pv", bufs=4, space="PSUM") as pvp,
        ):
            # resident loads, ordered/queued so iter-0's operands land first:
            # g quarter (d2=0, r<2) -> x[b=0] head -> m2 -> the rest
            xt_sb = []
            for b in range(NB):
                xb = rp.tile([128, XCOLS], bf16, tag=f"x{b}")
                xt_sb.append(xb)
            g_sb = rp.tile([128, GCOLS], bf16, tag="g")
            m2_sb = rp.tile([128, MCOLS], bf16, tag="m1")
            cut = 4 * (WT - 1) + 13          # x cols needed by chunk wc=0
            gq = GCOLS // 4
            nc.sync.dma_start(out=g_sb[:, 0:gq], in_=g[:, 0:gq])
            nc.sync.dma_start(out=xt_sb[0][:, 0:cut], in_=xT[0, :, 0:cut])
            nc.scalar.dma_start(out=m2_sb[:, :], in_=m1[:, :])
            nc.sync.dma_start(out=g_sb[:, gq:2 * gq], in_=g[:, gq:2 * gq])
            nc.scalar.dma_start(out=xt_sb[1][:, 0:cut], in_=xT[1, :, 0:cut])
            nc.sync.dma_start(out=g_sb[:, 2 * gq:], in_=g[:, 2 * gq:])
            nc.scalar.dma_start(out=xt_sb[0][:, cut:], in_=xT[0, :, cut:])
            nc.sync.dma_start(out=xt_sb[1][:, cut:], in_=xT[1, :, cut:])

            def emit_interp(d2, b, wc):
                """interp matmuls + ACT evac; returns dict (r,mt) -> y2."""
                w0 = WT * wc
                y2 = {}
                for r in range(4):
                    for mt in range(2):
                        py = pyp.tile([128, WT], f32, tag="py")
                        for s in range(NSLOT):
                            m = 4 * mt + s
                            base = 4 * w0 + m
                            rhs = xt_sb[b][:, base:base + 4 * WT:4]
                            nc.tensor.matmul(
                                py[:, :], g_sb[:, gcol(d2, r, mt, s)], rhs,
                                start=(s == 0), stop=(s == NSLOT - 1))
                        ysb = yp.tile([128, WT], bf16, tag="y")
                        if _VARIANT == "mmonly":
                            nc.scalar.copy(ysb[:, 0:1], py[:, 0:1])
                        else:
                            nc.scalar.copy(ysb[:, :], py[:, :])
                        y2[(r, mt)] = ysb
                return y2

            def emit_stage(d2, b, wc, y2):
                w0 = WT * wc
                # PSUM->SBUF evac engine per (r, pl): keep ACT under the PE
                # roofline by spreading 2 of 8 copies to DVE (Pool/GPSIMD
                # cannot read PSUM on hardware)
                evac_eng = {(0, 0): "a", (0, 1): "a", (1, 0): "v",
                            (1, 1): "a", (2, 0): "a", (2, 1): "a",
                            (3, 0): "v", (3, 1): "a"}
                for j in range(2):
                    ts = {}
                    for r in range(4):
                        for pl in range(2):
                            pv = pvp.tile([128, WT], f32, tag="pv")
                            for kt in range(2):
                                nc.tensor.matmul(
                                    pv[:, :], m2_sb[:, m2col(r, pl, kt, j)],
                                    y2[(r, kt)][:, :],
                                    start=(kt == 0), stop=(kt == 1))
                            tsb = tp.tile([128, WT], bf16, tag="t")
                            if _VARIANT == "mmonly":
                                nc.scalar.copy(tsb[:, 0:1], pv[:, 0:1])
                            else:
                                eng = evac_eng[(r, pl)]
                                if eng == "v":
                                    nc.vector.tensor_copy(tsb[:, :],
                                                          pv[:, :])
                                elif eng == "p":
                                    nc.gpsimd.tensor_copy(tsb[:, :],
                                                          pv[:, :])
                                else:
                                    nc.scalar.copy(tsb[:, :], pv[:, :])
                            ts[(r, pl)] = tsb
                    if _VARIANT in ("mmonly", "notail"):
                        continue
                    # multiply-free radix-4 combine, all bf16 SBUF (DVE 2x)
                    cmb = {}
                    for pl in range(2):
                        a = cp.tile([128, WT], bf16, tag="A")
                        nc.vector.tensor_add(a[:, :], ts[(0, pl)][:, :],
                                             ts[(2, pl)][:, :])
                        bb = cp.tile([128, WT], bf16, tag="B")
                        nc.vector.tensor_sub(bb[:, :], ts[(0, pl)][:, :],
                                             ts[(2, pl)][:, :])
                        c = cp.tile([128, WT], bf16, tag="C")
                        nc.vector.tensor_add(c[:, :], ts[(1, pl)][:, :],
                                             ts[(3, pl)][:, :])
                        u = cp.tile([128, WT], bf16, tag="U")
                        nc.vector.tensor_sub(u[:, :], ts[(1, pl)][:, :],
                                             ts[(3, pl)][:, :])
                        cmb[pl] = (a, bb, c, u)
                    are, bre, cre, ure = cmb[0]
                    aim, bim, cim, uim = cmb[1]
                    # out[q]: re plane at ot[:, 0:WT], im at ot[:, WT:2WT]
                    plan = {
                        0: ((are, cre, 1), (aim, cim, 1)),
                        2: ((are, cre, -1), (aim, cim, -1)),
                        1: ((bre, uim, 1), (bim, ure, -1)),
                        3: ((bre, uim, -1), (bim, ure, 1)),
                    }
                    for q in range(4):
                        ot = op.tile([128, 2 * WT], fp16, tag="o")
                        for pl in range(2):
                            x0, x1, sgn = plan[q][pl]
                            dst = ot[:, pl * WT:(pl + 1) * WT]
                            if sgn > 0:
                                nc.vector.tensor_add(dst, x0[:, :], x1[:, :])
                            else:
                                nc.vector.tensor_sub(dst, x0[:, :], x1[:, :])
                        if _VARIANT == "full":
                            kb = 256 * q + 128 * j
                            nc.sync.dma_start(
                                out=out_t[d2, b, kb:kb + 128, :,
                                          w0:w0 + WT],
                                in_=ot[:, :].rearrange("p (r w) -> p r w",
                                                       r=2))

            import contextlib
            _hints = ()
            if _os.environ.get("LOOP_HINTS"):
                _hints = (mybir.EngineType.PE, mybir.EngineType.Activation,
                          mybir.EngineType.DVE, mybir.EngineType.SP)
            rep_ctx = (tc.For_i(0, _REPEAT, 1, hint_engines=_hints)
                       if _REPEAT > 1 else contextlib.nullcontext())
            with rep_ctx:
                # software pipeline: interp(i+1) issues before stage(i) so
                # the PE never waits on the interp->ACT-evac->stage chain
                iters = [(d2, b, wc) for d2 in range(DLOC)
                         for b in range(NB) for wc in range(NWC)]
                pending = emit_interp(*iters[0])
                for i, it in enumerate(iters):
                    nxt = (emit_interp(*iters[i + 1])
                           if i + 1 < len(iters) else None)
                    emit_stage(*it, pending)
                    pending = nxt
            if tick is not None:
                nc.sync.dma_start(out=tick[:, :], in_=g_sb[:, 0:1])
    nc.compile()
    return nc


def _host_prep(x, dlnf):
    x = np.ascontiguousarray(np.asarray(x, dtype=np.float32))
    dlnf = np.asarray(dlnf, dtype=np.float32)
    # x shifted by -64 into partition-interleaved layout:
    # xT[b, q, c] = x[b, 128 c + q - 64]  (zeros outside [0, NX))
    xT = np.zeros((NB, 128, XCOLS), dtype=np.float32)
    xs = np.zeros((NB, XCOLS * 128), dtype=np.float32)
    xs[:, 64:64 + NX] = x
    xT[:, :, :] = np.transpose(xs.reshape(NB, XCOLS, 128), (0, 2, 1))
    xT = xT.astype(ml_dtypes.bfloat16)
    m2 = _build_m2().astype(ml_dtypes.bfloat16)
    lo_all, frac_all = _host_tables_all(dlnf)
    in_maps = []
    for c in range(NCORES):
        gc_ = _build_g(lo_all[DLOC * c: DLOC * (c + 1)],
                       frac_all[DLOC * c: DLOC * (c + 1)])
        in_maps.append({"xT": xT, "g": gc_.astype(ml_dtypes.bfloat16),
                        "m1": m2})
    return in_maps


def kernel(x, dlnf):
    from concourse.bass_utils import run_bass_kernel_spmd

    in_maps = _host_prep(x, dlnf)
    if "nc" not in _NC_CACHE:
        _NC_CACHE["nc"] = _build_program()
    nc = _NC_CACHE["nc"]
    res = run_bass_kernel_spmd(nc, in_maps, core_ids=list(range(NCORES)))
    _LAST_RESULTS["res"] = res
    outs = []
    for c in range(NCORES):
        o = np.asarray(res.results[c]["out"], dtype=np.float32)
        # [DLOC, NB, K, 2, 1024] fp16 planes -> complex64 [DLOC, NB, W, K]
        cplx = (o[:, :, :, 0, :W] + 1j * o[:, :, :, 1, :W]).astype(
            np.complex64)
        outs.append(np.transpose(cplx, (0, 1, 3, 2)))
    return np.concatenate(outs, axis=0)


# revision 39
# speedup vs baseline: 81743.9706x; 66566.9252x over previous
"""DechirpSTFT Trainium2 kernel (8 NeuronCores).

Math: out[d,b,w,:] = FFT_1024(chirp * resample_d(hann * window(x[b], w)))

Per (d, b), radix-4 DIT with every twiddle folded into a stationary:
  - window + hann + linear-interp resample -> banded matrix G_d applied by
    TensorE to x held in SBUF as [128, 4104] bf16 (window = stride-4 column
    slice; hop 512 = 4 cols of 128).  G emits z-tiles (r, mt) holding
    y[4*(128*mt + p) + r]; each needs 5 source-tile slots (n-span 512).
  - stage: T_r[u] = sum_m chirp[4m+r] W1024^{r u} W256^{m u} y[4m+r],
    u in [0,256) -- four 256-point DFTs (chirp + r-twiddle folded into the
    stationaries), TensorE, 2 real planes, contraction 256 = 2 k-tiles.
  - combine (multiply-free, DVE bf16): A=T0+T2, B=T0-T2, C=T1+T3, U=T1-T3;
    out[q=0]=A+C, out[q=2]=A-C, out[q=1]=(Bre+Uim, Bim-Ure),
    out[q=3]=(Bre-Uim, Bim+Ure), written as packed fp16 re/im planes
    (host interleaves to complex64).  72 matmuls/iter vs 88 for the
    radix-2 dense-512 scheme.
Each core owns 2 of the 16 chirp rates.
"""

import os as _os

import ml_dtypes
import numpy as np

K = 1024
HOP = 512
CHIRP_A = 0.5
NB = 2
NX = 524288
W = (NX - K) // HOP + 1          # 1023
D = 16
NCORES = 8
DLOC = D // NCORES               # 2 chirp rates per core
WT = 512                          # windows per chunk (matmul moving dim)
NWC = 2                           # ceil(1023/512)
NSLOT = 5                         # interp source tiles per z-tile
XPH = 1026                        # x cols per phase (4-way de-interleaved)
XCOLS = 4 * XPH                   # 4104: 4096 cols + pad for window 1023
GCOLS = DLOC * 4 * 2 * NSLOT * 128
MCOLS = 4 * 2 * 2 * 2 * 128

_NC_CACHE = {}
_LAST_RESULTS = {}
_REPEAT = 1  # >1: wrap body in a device-side loop (timing experiments only)
_VARIANT = "full"  # timing-only: full | nodma | notail | mmonly
_TIMING = False  # timing-only: out stays in DRAM (Internal); tiny tick output


def _host_tables_all(dlnf):
    """(16,) -> lo (D,K) int32, frac (D,K) f32.  Computed with jax on CPU,
    bit-exactly mirroring reference.py's fp32 pipeline (numpy's fp32
    exp/log1p differ from XLA's by enough to shift idx by ~1e-3 samples)."""
    import jax
    import jax.numpy as jnp

    cpu = jax.devices("cpu")[0]
    with jax.default_device(cpu):
        betas = 2.0 * jnp.asarray(np.asarray(dlnf, dtype=np.float32))
        safe = jnp.abs(betas) < 1e-8
        bs = jnp.where(safe, jnp.float32(1e-8), betas)
        tau = jnp.linspace(0.0, 1.0, K, dtype=jnp.float32)
        t_src = 2.0 / bs[:, None] * jnp.log1p(
            tau[None, :] * (jnp.exp(bs)[:, None] - 1.0)) - 1.0
        identity = jnp.linspace(-1.0, 1.0, K, dtype=jnp.float32)
        t_src = jnp.where(safe[:, None], identity[None, :], t_src)
        idx = (t_src + 1.0) * 0.5 * (K - 1)
        lo = jnp.clip(idx.astype(jnp.int32), 0, K - 2)
        frac = idx - lo.astype(idx.dtype)
    return np.asarray(lo), np.asarray(frac).astype(np.float32)


def _build_g(lo_pair, frac_pair):
    """Interp stationaries, packed [128, DLOC*4*2*5*128] fp32.
    Col block ((d2*4+r)*2+mt)*5+s holds G[q, p]:
    src x[128*(4mt+s) + q - 64]  ->  y[4*(128*mt + p) + r]."""
    hann = (0.5 * (1.0 - np.cos(2.0 * np.pi * np.arange(K) / K))).astype(
        np.float32)
    g = np.zeros((128, GCOLS), dtype=np.float32)
    nn = np.arange(K)
    r_, mh = nn & 3, nn >> 2
    mt_, p_ = mh >> 7, mh & 127
    for d2 in range(DLOC):
        lo = lo_pair[d2]
        frac = frac_pair[d2]
        alpha = ((1.0 - frac) * hann[lo]).astype(np.float32)
        beta = (frac * hann[lo + 1]).astype(np.float32)
        for j, val in ((lo, alpha), (lo + 1, beta)):
            m, q = (j + 64) >> 7, (j + 64) & 127
            s = m - 4 * mt_
            if not np.all((s >= 0) & (s < NSLOT)):
                raise ValueError("interp band exceeds the source-tile slots")
            flat = ((d2 * 4 + r_) * 2 + mt_) * NSLOT + s
            np.add.at(g, (q, flat * 128 + p_), val)
    return g


def _build_m2():
    """Stage stationaries [128, 4*2*2*2*128] fp32 (d-independent).
    Col block ((r*2+pl)*2+kt)*2+j holds plane pl of
    M_r[m, u] = chirp[4m+r] * W256^{m u} * W1024^{r u},
    rows m = 128*kt + row, cols u = 128*j + col."""
    t_norm = np.linspace(-1.0, 1.0, K).astype(np.float64)
    chirp = np.exp(-1j * CHIRP_A * t_norm ** 2)
    m2 = np.zeros((128, MCOLS), dtype=np.float32)
    mg = np.arange(256)
    ug = np.arange(256)
    for r in range(4):
        M = (chirp[4 * mg + r][:, None]
             * np.exp(-2j * np.pi * np.outer(mg, ug) / 256.0)
             * np.exp(-2j * np.pi * r * ug / 1024.0)[None, :])
        for pl in range(2):
            plane = (M.real if pl == 0 else M.imag).astype(np.float32)
            for kt in range(2):
                for j in range(2):
                    flat = ((r * 2 + pl) * 2 + kt) * 2 + j
                    m2[:, flat * 128:(flat + 1) * 128] = \
                        plane[128 * kt:128 * kt + 128, 128 * j:128 * j + 128]
    return m2


def _build_program():
    import concourse.bacc as bacc
    import concourse.mybir as mybir
    from concourse.tile import TileContext

    f32 = mybir.dt.float32
    bf16 = mybir.dt.bfloat16
    fp16 = mybir.dt.float16

    nc = bacc.Bacc("TRN2", target_bir_lowering=False, debug=False,
                   num_devices=NCORES)
    xT = nc.dram_tensor("xT", [NB, 128, 4, XPH], bf16, kind="ExternalInput")
    g = nc.dram_tensor("g", [128, GCOLS], bf16, kind="ExternalInput")
    m1 = nc.dram_tensor("m1", [128, MCOLS], bf16, kind="ExternalInput")
    # output: separate re/im planes, W padded to 1024 so DMAs are full-tile
    out_t = nc.dram_tensor("out", [DLOC, NB, K, 2, 1024], fp16,
                           kind="Internal" if _TIMING else "ExternalOutput")
    tick = (nc.dram_tensor("tick", [128, 1], bf16, kind="ExternalOutput")
            if _TIMING else None)

    def gcol(d2, r, mt, s):
        flat = ((d2 * 4 + r) * 2 + mt) * NSLOT + s
        return slice(flat * 128, (flat + 1) * 128)

    def m2col(r, pl, kt, j):
        flat = ((r * 2 + pl) * 2 + kt) * 2 + j
        return slice(flat * 128, (flat + 1) * 128)

    with TileContext(nc) as tc:
        with (
            tc.tile_pool(name="resident", bufs=1) as rp,
            tc.tile_pool(name="ysb", bufs=18) as yp,
            tc.tile_pool(name="tsb", bufs=12) as tp,
            tc.tile_pool(name="csb", bufs=10) as cp,
            tc.tile_pool(name="osb", bufs=4) as op,
            tc.tile_pool(name="py", bufs=2, space="PSUM") as pyp,
            tc.tile_pool(name="pv", bufs=3, space="PSUM") as pvp,
        ):
            # resident loads, ordered/queued so iter-0's operands land first:
            # g quarter (d2=0, r<2) -> x[b=0] head -> m2 -> the rest
            xt_sb = []
            for b in range(NB):
                xb = rp.tile([128, XCOLS], bf16, tag=f"x{b}")
                xt_sb.append(xb)
            g_sb = rp.tile([128, GCOLS], bf16, tag="g")
            m2_sb = rp.tile([128, MCOLS], bf16, tag="m1")
            # xb layout: [128, 4*XPH], phase-major: col ph*XPH + c holds
            # x[128*(4c+ph) + q - 64]; chunk wc=0 needs c < cut per phase
            cut = WT + 4
            gq = GCOLS // 4
            g0 = NSLOT * 128                 # blocks for z-tile (r=0, mt=0)
            nc.sync.dma_start(out=g_sb[:, 0:g0], in_=g[:, 0:g0])
            nc.sync.dma_start(out=g_sb[:, g0:gq], in_=g[:, g0:gq])
            nc.sync.dma_start(
                out=xt_sb[0][:, :].rearrange("p (h c) -> p h c",
                                             h=4)[:, :, 0:cut],
                in_=xT[0, :, :, 0:cut])
            nc.scalar.dma_start(out=m2_sb[:, :], in_=m1[:, :])
            nc.sync.dma_start(out=g_sb[:, gq:2 * gq], in_=g[:, gq:2 * gq])
            nc.scalar.dma_start(
                out=xt_sb[1][:, :].rearrange("p (h c) -> p h c",
                                             h=4)[:, :, 0:cut],
                in_=xT[1, :, :, 0:cut])
            nc.sync.dma_start(out=g_sb[:, 2 * gq:], in_=g[:, 2 * gq:])
            nc.scalar.dma_start(
                out=xt_sb[0][:, :].rearrange("p (h c) -> p h c",
                                             h=4)[:, :, cut:],
                in_=xT[0, :, :, cut:])
            nc.sync.dma_start(
                out=xt_sb[1][:, :].rearrange("p (h c) -> p h c",
                                             h=4)[:, :, cut:],
                in_=xT[1, :, :, cut:])

            def emit_interp(d2, b, wc):
                """interp matmuls + ACT evac; returns dict (r,mt) -> y2."""
                w0 = WT * wc
                y2 = {}
                for r in range(4):
                    for mt in range(2):
                        py = pyp.tile([128, WT], f32, tag="py")
                        for s in range(NSLOT):
                            m = 4 * mt + s
                            ph, c0 = m & 3, m >> 2
                            base = XPH * ph + c0 + w0
                            rhs = xt_sb[b][:, base:base + WT]
                            nc.tensor.matmul(
                                py[:, :], g_sb[:, gcol(d2, r, mt, s)], rhs,
                                start=(s == 0), stop=(s == NSLOT - 1))
                        ysb = yp.tile([128, WT], bf16, tag="y")
                        if _VARIANT == "mmonly":
                            nc.scalar.copy(ysb[:, 0:1], py[:, 0:1])
                        else:
                            nc.scalar.copy(ysb[:, :], py[:, :])
                        y2[(r, mt)] = ysb
                return y2

            def emit_stage(d2, b, wc, y2, last=False):
                w0 = WT * wc
                # j-merged: pv [128, 2*WT] holds both u-tiles (j) side by
                # side; evac/combine ops run at [128, 1024] to halve op
                # count.  PSUM->SBUF evac engine per (r, pl): 2 of 8 on DVE
                # (Pool/GPSIMD cannot read PSUM on hardware).
                evac_eng = {(0, 0): "a", (0, 1): "a", (1, 0): "a",
                            (1, 1): "a", (2, 0): "a", (2, 1): "a",
                            (3, 0): "a", (3, 1): "a"}
                ts = {}
                # last iter: j-split pv + r-order 0,2,1,3 so j=0's combine
                # overlaps j=1's matmuls, shortening the final drain
                jgroups = ((0,), (1,)) if last else ((0, 1),)
                for jg in jgroups:
                    for r in ((0, 2, 1, 3) if last else range(4)):
                        for pl in range(2):
                            pv = pvp.tile([128, len(jg) * WT], f32,
                                          tag="pv")
                            for ji, j in enumerate(jg):
                                dst = pv[:, ji * WT:(ji + 1) * WT]
                                for kt in range(2):
                                    nc.tensor.matmul(
                                        dst, m2_sb[:, m2col(r, pl, kt, j)],
                                        y2[(r, kt)][:, :],
                                        start=(kt == 0), stop=(kt == 1))
                            tsb = tp.tile([128, len(jg) * WT], bf16,
                                          tag="t")
                            if _VARIANT == "mmonly":
                                nc.scalar.copy(tsb[:, 0:1], pv[:, 0:1])
                            elif evac_eng[(r, pl)] == "v":
                                nc.vector.tensor_copy(tsb[:, :], pv[:, :])
                            else:
                                nc.scalar.copy(tsb[:, :], pv[:, :])
                            ts[(r, pl, jg[0])] = tsb
                if _VARIANT in ("mmonly", "notail"):
                    return
                # multiply-free radix-4 combine, all bf16 SBUF (DVE 2x)
                for jh in ((0,), (1,)) if last else ((0, 1),):
                    lo = jh[0] * WT
                    hi = (jh[-1] + 1) * WT
                    sl = slice(0, hi - lo) if last else slice(lo, hi)
                    jk = jh[0] if last else 0

                    def tsv(r, pl):
                        return ts[(r, pl, jk)][:, sl]

                    cmb = {}
                    for pl in range(2):
                        a = cp.tile([128, hi - lo], bf16, tag="A")
                        nc.vector.tensor_add(a[:, :], tsv(0, pl),
                                             tsv(2, pl))
                        bb = cp.tile([128, hi - lo], bf16, tag="B")
                        nc.vector.tensor_sub(bb[:, :], tsv(0, pl),
                                             tsv(2, pl))
                        c = cp.tile([128, hi - lo], bf16, tag="C")
                        nc.vector.tensor_add(c[:, :], tsv(1, pl),
                                             tsv(3, pl))
                        u = cp.tile([128, hi - lo], bf16, tag="U")
                        nc.vector.tensor_sub(u[:, :], tsv(1, pl),
                                             tsv(3, pl))
                        cmb[pl] = (a, bb, c, u)
                    are, bre, cre, ure = cmb[0]
                    aim, bim, cim, uim = cmb[1]
                    # ot cols: pl*(hi-lo) + w; out k = 256q + 128j + p
                    plan = {
                        0: ((are, cre, 1), (aim, cim, 1)),
                        2: ((are, cre, -1), (aim, cim, -1)),
                        1: ((bre, uim, 1), (bim, ure, -1)),
                        3: ((bre, uim, -1), (bim, ure, 1)),
                    }
                    njh = len(jh)
                    for q in range(4):
                        ot = op.tile([128, njh * 2 * WT], fp16, tag="o")
                        for pl in range(2):
                            x0, x1, sgn = plan[q][pl]
                            dst = ot[:, pl * njh * WT:(pl + 1) * njh * WT]
                            if sgn > 0:
                                nc.vector.tensor_add(dst, x0[:, :],
                                                     x1[:, :])
                            else:
                                nc.vector.tensor_sub(dst, x0[:, :],
                                                     x1[:, :])
                        if _VARIANT == "full":
                            for jj in range(njh):
                                j = jh[jj]
                                kb = 256 * q + 128 * j
                                # alternate queues so transfers parallelize
                                dma_eng = nc.sync if (q + j) % 2 == 0 \
                                    else nc.scalar
                                dma_eng.dma_start(
                                    out=out_t[d2, b, kb:kb + 128, :,
                                              w0:w0 + WT],
                                    in_=ot[:, :].rearrange(
                                        "p (r j w) -> p j r w",
                                        r=2, j=njh)[:, jj, :, :])

            import contextlib
            _hints = ()
            if _os.environ.get("LOOP_HINTS"):
                _hints = (mybir.EngineType.PE, mybir.EngineType.Activation,
                          mybir.EngineType.DVE, mybir.EngineType.SP)
            rep_ctx = (tc.For_i(0, _REPEAT, 1, hint_engines=_hints)
                       if _REPEAT > 1 else contextlib.nullcontext())
            with rep_ctx:
                # software pipeline: interp(i+1) issues before stage(i) so
                # the PE never waits on the interp->ACT-evac->stage chain
                iters = [(d2, b, wc) for d2 in range(DLOC)
                         for b in range(NB) for wc in range(NWC)]
                pending = emit_interp(*iters[0])
                for i, it in enumerate(iters):
                    nxt = (emit_interp(*iters[i + 1])
                           if i + 1 < len(iters) else None)
                    emit_stage(*it, pending, last=(i + 1 == len(iters)))
                    pending = nxt
            if tick is not None:
                nc.sync.dma_start(out=tick[:, :], in_=g_sb[:, 0:1])
    nc.compile()
    return nc


def _host_prep(x, dlnf):
    x = np.ascontiguousarray(np.asarray(x, dtype=np.float32))
    dlnf = np.asarray(dlnf, dtype=np.float32)
    # x shifted by -64, partition-interleaved and 4-way phase-split:
    # xT[b, q, ph, c] = x[b, 128*(4c+ph) + q - 64]  (zeros outside [0, NX))
    xs = np.zeros((NB, XCOLS * 128), dtype=np.float32)
    xs[:, 64:64 + NX] = x
    xT = np.ascontiguousarray(np.transpose(
        xs.reshape(NB, XPH, 4, 128), (0, 3, 2, 1))).astype(
        ml_dtypes.bfloat16)
    m2 = _build_m2().astype(ml_dtypes.bfloat16)
    lo_all, frac_all = _host_tables_all(dlnf)
    in_maps = []
    for c in range(NCORES):
        gc_ = _build_g(lo_all[DLOC * c: DLOC * (c + 1)],
                       frac_all[DLOC * c: DLOC * (c + 1)])
        in_maps.append({"xT": xT, "g": gc_.astype(ml_dtypes.bfloat16),
                        "m1": m2})
    return in_maps


def kernel(x, dlnf):
    from concourse.bass_utils import run_bass_kernel_spmd

    in_maps = _host_prep(x, dlnf)
    if "nc" not in _NC_CACHE:
        _NC_CACHE["nc"] = _build_program()
    nc = _NC_CACHE["nc"]
    res = run_bass_kernel_spmd(nc, in_maps, core_ids=list(range(NCORES)))
    _LAST_RESULTS["res"] = res
    outs = []
    for c in range(NCORES):
        o = np.asarray(res.results[c]["out"], dtype=np.float32)
        # [DLOC, NB, K, 2, 1024] fp16 planes -> complex64 [DLOC, NB, W, K]
        cplx = (o[:, :, :, 0, :W] + 1j * o[:, :, :, 1, :W]).astype(
            np.complex64)
        outs.append(np.transpose(cplx, (0, 1, 3, 2)))
    return np.concatenate(outs, axis=0)
